# revision 1
# baseline (speedup 1.0000x reference)
"""Trainium2 Bass kernel for the custom MHA problem nn_CustomMHA_14551349198906.

Computation (per batch b):
    t = x @ w_qkv.T ; q,k,v = heads of t        # (S, 3D), H=16 heads of 64
    scores = einsum('sid,sjd->sij', q, k)/sqrt(D)   # per-token 16x16
    lower-tri mask, exact-0 -> -inf, softmax over j
    y' = p @ v ; out = y' @ w_o.T

Distribution: pure data-parallel over the batch (B=8 -> one NeuronCore per
batch element); no collectives.

Per-core program (Tile framework), per 128-token tile:
  - PE computes t via out[s,o] psum tiles (stationary = transposed x block,
    moving = transposed w_qkv);  q/k via 3-pass bf16 split matmuls
    (hi*hi + hi*lo + lo*hi, ~1e-5 relative - softmax logits need accuracy),
    v and the output matmul via single-pass float32r (~1.5e-4 relative).
  - The per-token 16x16 head attention cannot use the PE (both operands
    vary per token), so it runs on the Vector engine with tokens on
    partitions: one plain-strided multiply per diagonal (i-j) + segmented
    reduce builds the scores into a -3e38-padded [128,16,18] rect
    (padding exp -> 0), and p@v multiplies in bf16 with a per-token-
    transposed v layout so both operands stream unit-stride at the 2x
    DVE mode.  (GpSimd offload was measured slower: shared SBUF port +
    per-row cross-engine ping-pong.)
  - y' is PE-transposed (identity matmul) to feed the w_o matmul.

The local walrus build encodes at most ONE inline sync-wait per TPB
instruction; split_excess_waits() hoists extra waits onto same-engine NOPs.
"""

import math
from contextlib import ExitStack

import numpy as np

import bass_rust
import concourse.bass as bass
import concourse.mybir as mybir
import concourse.tile as tile
from concourse.masks import make_identity

F32 = mybir.dt.float32
BF16 = mybir.dt.bfloat16
F32R = mybir.dt.float32r

B, S, D, H, DH = 8, 4096, 1024, 16, 64
TILE = 128
KB = D // 128
N_TILES = S // TILE
NEG = -3.0e38
N_CORES = 8
W = 18   # padded softmax row width

CFG = {
    "qk": "bf16x3",      # q/k matmul: bf16 hi/lo 3-pass split
    "v": "f32r",         # v matmul: single-pass float32r
    "pv": "bf16",        # middle p@v leg in bf16
    "out_mm": "f32r",    # w_o matmul: float32r
    "pool_rows": 0,      # GpSimd offload measured slower (shared SBUF port,
    "pv_pool_rows": 0,   # per-row cross-engine ping-pong) - keep all on DVE
    "bufs": 2,
}


# --------------------------------------------------------------------------
# walrus workaround: hoist excess sync waits onto same-engine NOPs
# --------------------------------------------------------------------------
def split_excess_waits(nc, max_waits=1):
    n_split = 0
    for fn in nc.m.functions:
        for bb in fn.blocks:
            out = []
            changed = False
            for inst in bb.instructions:
                si = inst.sync_info
                waits = list(si.on_wait) if si is not None and si.on_wait else []
                if len(waits) > max_waits:
                    reg = [w for w in waits if getattr(w, "wait_reg", None) is not None]
                    imm = [w for w in waits if getattr(w, "wait_reg", None) is None]
                    kept = reg[:]
                    hoist = []
                    for w in imm:
                        if len(kept) < max_waits:
                            kept.append(w)
                        else:
                            hoist.append(w)
                    if len(kept) > max_waits:
                        raise RuntimeError(
                            f"{inst.name}: {len(reg)} register waits exceed limit")
                    for j, w in enumerate(hoist):
                        nop = mybir.InstNoOp(
                            name=f"{inst.name}-wsplit{j}",
                            ins=[], outs=[], engine=inst.engine)
                        nop.sync_info = bass_rust.SyncInfo(
                            on_wait=[w], on_update=[])
                        out.append(nop)
                        n_split += 1
                    inst.sync_info = bass_rust.SyncInfo(
                        on_wait=kept,
                        on_update=list(si.on_update) if si.on_update else [])
                    changed = True
                out.append(inst)
            if changed:
                bb.instructions = out
    return n_split


# --------------------------------------------------------------------------
# device program
# --------------------------------------------------------------------------
def _rearr_kb(ap):
    return ap.rearrange("(kb p) n -> p kb n", p=128)


def build(nc, cfg):
    qk = cfg["qk"]
    pv_dt = BF16 if cfg["pv"] == "bf16" else F32
    out_mm = cfg["out_mm"]
    pool_rows = cfg["pool_rows"]
    pv_pool_rows = cfg["pv_pool_rows"]
    bufs = cfg["bufs"]

    # float32r DRAM declarations carry the same f32 bits; the PE rounds on
    # read (verified bit-identical to an explicit rounding copy).
    w_dt = F32R
    x_dt = F32R
    wo_dt = F32R if out_mm == "f32r" else F32

    xT = nc.dram_tensor("xT", (D, S), x_dt, kind="ExternalInput").ap()
    wq = nc.dram_tensor("wq", (D, 3 * D), w_dt, kind="ExternalInput").ap()
    wo = nc.dram_tensor("wo", (D, D), wo_dt, kind="ExternalInput").ap()
    if qk == "bf16x3":
        xTh = nc.dram_tensor("xTh", (D, S), BF16, kind="ExternalInput").ap()
        xTl = nc.dram_tensor("xTl", (D, S), BF16, kind="ExternalInput").ap()
        wqh = nc.dram_tensor("wqh", (D, 2 * D), BF16, kind="ExternalInput").ap()
        wql = nc.dram_tensor("wql", (D, 2 * D), BF16, kind="ExternalInput").ap()
    out = nc.dram_tensor("out", (S, D), F32, kind="ExternalOutput").ap()

    out_dt = {"f32": F32, "f32r": F32R}[out_mm]

    with tile.TileContext(nc) as tc, ExitStack() as ctx:
        wpool = ctx.enter_context(tc.tile_pool(name="w", bufs=1))
        cpool = ctx.enter_context(tc.tile_pool(name="const", bufs=1))
        xpool = ctx.enter_context(tc.tile_pool(name="x", bufs=bufs + 1))
        tpool = ctx.enter_context(tc.tile_pool(name="t", bufs=bufs))
        spool = ctx.enter_context(tc.tile_pool(name="sm", bufs=bufs))
        ypool = ctx.enter_context(tc.tile_pool(name="y", bufs=bufs))
        opool = ctx.enter_context(tc.tile_pool(name="o", bufs=bufs))
        scratch = ctx.enter_context(tc.tile_pool(name="scr", bufs=1))
        pp_t = ctx.enter_context(tc.tile_pool(name="ps_t", bufs=4, space="PSUM"))
        pp_tr = ctx.enter_context(tc.tile_pool(name="ps_tr", bufs=2, space="PSUM"))
        pp_o = ctx.enter_context(tc.tile_pool(name="ps_o", bufs=2, space="PSUM"))

        if qk == "f32r":
            wq_sb = wpool.tile([128, KB, 3 * D], w_dt)
            nc.sync.dma_start(wq_sb[:], _rearr_kb(wq))
            wq_qk, wqv_sb, v_off = wq_sb, wq_sb, 2 * D
        else:
            wqv_sb = wpool.tile([128, KB, D], w_dt)
            nc.sync.dma_start(wqv_sb[:], _rearr_kb(wq[:, 2 * D:3 * D]))
            v_off = 0
            wqh_sb = wpool.tile([128, KB, 2 * D], BF16)
            nc.sync.dma_start(wqh_sb[:], _rearr_kb(wqh))
            wql_sb = wpool.tile([128, KB, 2 * D], BF16)
            nc.sync.dma_start(wql_sb[:], _rearr_kb(wql))

        wo_t = wpool.tile([128, KB, D], wo_dt)
        nc.sync.dma_start(wo_t[:], _rearr_kb(wo))

        ident = cpool.tile([128, 128], pv_dt)
        make_identity(nc, ident[:])

        for n in range(N_TILES):
            sl = slice(n * TILE, (n + 1) * TILE)

            xt = xpool.tile([128, KB, TILE], x_dt, tag="xt")
            nc.sync.dma_start(
                xt[:], xT[:, sl].rearrange("(kb p) s -> p kb s", p=128))
            if qk == "bf16x3":
                xth = xpool.tile([128, KB, TILE], BF16, tag="xth")
                nc.sync.dma_start(
                    xth[:], xTh[:, sl].rearrange("(kb p) s -> p kb s", p=128))
                xtl = xpool.tile([128, KB, TILE], BF16, tag="xtl")
                nc.sync.dma_start(
                    xtl[:], xTl[:, sl].rearrange("(kb p) s -> p kb s", p=128))

            # ---- step 1: t = x @ w_qkv.T into 6 psum chunks -------------
            ps_t = []
            for oc in range(6):
                ps = pp_t.tile([128, 512], F32, tag="t", name=f"pst{n}_{oc}")
                o0 = oc * 512
                if oc < 4:
                    if qk == "f32r":
                        passes = [(xt, wq_qk, o0)]
                    else:
                        passes = [(xth, wqh_sb, o0), (xth, wql_sb, o0),
                                  (xtl, wqh_sb, o0)]
                else:
                    passes = [(xt, wqv_sb, v_off + (oc - 4) * 512)]
                np_ = len(passes)
                for kb in range(KB):
                    for pi, (a, b, off) in enumerate(passes):
                        nc.tensor.matmul(
                            ps[:], a[:, kb, :], b[:, kb, off:off + 512],
                            start=(kb == 0 and pi == 0),
                            stop=(kb == KB - 1 and pi == np_ - 1))
                ps_t.append(ps)

            # ---- drains -------------------------------------------------
            q_sb = tpool.tile([128, D], F32, tag="q")
            k_sb = tpool.tile([128, D], F32, tag="k")
            vt = tpool.tile([128, D], pv_dt, tag="vt")
            nc.scalar.copy(q_sb[:, 0:512], ps_t[0][:])
            nc.scalar.copy(q_sb[:, 512:1024], ps_t[1][:])
            nc.scalar.copy(k_sb[:, 0:512], ps_t[2][:])
            nc.scalar.copy(k_sb[:, 512:1024], ps_t[3][:])
            # vt[:, 16*d + j] = v[:, 64*j + d]
            for h in range(2):
                src = ps_t[4 + h][:].rearrange("p (j d) -> p j d", d=DH)
                dst = vt[:].rearrange("p (d j) -> p d j", j=H)[
                    :, :, h * 8:(h + 1) * 8].rearrange("p d j -> p j d")
                nc.scalar.copy(dst, src)

            # ---- scores: one multiply per diagonal (i-j = o) ------------
            # plain stride-64 APs on both operands (no step-0 broadcast);
            # scores land at stride W+1 in a width-W=18 -3e38-padded rect
            # (W=18 keeps bf16 row starts 4B-aligned for the p@v 2x mode)
            screct = spool.tile([128, H * W], F32, tag="sc")
            nc.gpsimd.memset(screct[:], NEG)
            prod = scratch.tile([128, D], F32, tag="prod")
            q3 = q_sb[:].rearrange("p (i d) -> p i d", d=DH)
            k3 = k_sb[:].rearrange("p (j d) -> p j d", d=DH)
            for o in range(H):
                cnt = H - o
                pr3 = prod[:, 0:cnt * DH].rearrange("p (j d) -> p j d", d=DH)
                nc.vector.tensor_mul(pr3, q3[:, o:H, :], k3[:, 0:cnt, :])
                nc.vector.reduce_sum(
                    screct[:, W * o:W * o + (W + 1) * (cnt - 1) + 1:W + 1],
                    pr3, axis=mybir.AxisListType.X)

            # ---- softmax over j -----------------------------------------
            sc3 = screct[:].rearrange("p (i j) -> p i j", j=W)
            mrow = spool.tile([128, H], F32, tag="m")
            nc.vector.reduce_max(mrow[:], sc3, axis=mybir.AxisListType.X)
            mb = mrow[:].rearrange("p (i one) -> p i one", one=1).broadcast_to(
                (128, H, W))
            shifted = spool.tile([128, H * W], F32, tag="sh")
            nc.vector.tensor_sub(
                shifted[:].rearrange("p (i j) -> p i j", j=W), sc3, mb)
            ebuf = spool.tile([128, H * W], F32, tag="e")
            nc.scalar.activation(ebuf[:], shifted[:],
                                 mybir.ActivationFunctionType.Exp)
            zrow = spool.tile([128, H], F32, tag="z")
            nc.vector.reduce_sum(
                zrow[:], ebuf[:].rearrange("p (i j) -> p i j", j=W),
                axis=mybir.AxisListType.X)
            rrow = spool.tile([128, H], F32, tag="r")
            nc.vector.reciprocal(rrow[:], zrow[:])
            prect = spool.tile([128, H * W], pv_dt, tag="pr")
            rb = rrow[:].rearrange("p (i one) -> p i one", one=1).broadcast_to(
                (128, H, W))
            nc.vector.tensor_mul(
                prect[:].rearrange("p (i j) -> p i j", j=W),
                ebuf[:].rearrange("p (i j) -> p i j", j=W), rb)

            # ---- p @ v --------------------------------------------------
            yp = ypool.tile([128, D], pv_dt, tag="yp")
            pvprod = scratch.tile([128, D], pv_dt, tag="pvp")
            p3 = prect[:].rearrange("p (i j) -> p i j", j=W)[:, :, 0:H]
            vt3 = vt[:].rearrange("p (d j) -> p d j", j=H)
            for i in range(H):
                # even j-count keeps the bf16 2x DVE mode; padded p col is 0
                nj = min(H, (i + 2) & ~1) if pv_dt == BF16 else i + 1
                pb = p3[:, i:i + 1, 0:nj].broadcast_to((128, DH, nj))
                pp3 = pvprod[:, 0:DH * nj].rearrange("p (d j) -> p d j", j=nj)
                eng = nc.gpsimd if i < pv_pool_rows else nc.vector
                eng.tensor_mul(pp3, pb, vt3[:, :, 0:nj])
                with nc.allow_low_precision(reason="p@v sums <=16 bf16 terms"):
                    nc.vector.reduce_sum(
                        yp[:].rearrange("p (i d) -> p i d", d=DH)[:, i, :],
                        pp3, axis=mybir.AxisListType.X)

            # ---- transpose y', then out = y' @ w_o.T --------------------
            ypT = ypool.tile([128, KB, TILE], out_dt, tag="ypT")
            ps_tr = [pp_tr.tile([128, 512], pv_dt, tag="tr", name=f"tr{n}_{h}")
                     for h in range(2)]
            for kb in range(KB):
                nc.tensor.transpose(
                    ps_tr[kb // 4][:, (kb % 4) * 128:(kb % 4 + 1) * 128],
                    yp[:, kb * 128:(kb + 1) * 128], ident[:])
            for h in range(2):
                nc.scalar.copy(
                    ypT[:].rearrange("p kb s -> p (kb s)")[
                        :, h * 512:(h + 1) * 512], ps_tr[h][:])

            osb = opool.tile([128, D], F32, tag="osb")
            for oc in range(2):
                ps_o = pp_o.tile([128, 512], F32, tag="o", name=f"pso{n}_{oc}")
                for kb in range(KB):
                    nc.tensor.matmul(
                        ps_o[:], ypT[:, kb, :], wo_t[:, kb, oc * 512:(oc + 1) * 512],
                        start=(kb == 0), stop=(kb == KB - 1))
                nc.scalar.copy(osb[:, oc * 512:(oc + 1) * 512], ps_o[:])
            nc.sync.dma_start(out[sl, :], osb[:])

    return nc


# --------------------------------------------------------------------------
# host side
# --------------------------------------------------------------------------
_CACHE = {}


def _get_nc():
    if "nc" not in _CACHE:
        nc = bass.Bass("TRN2", target_bir_lowering=False, debug=False,
                       num_devices=N_CORES)
        build(nc, CFG)
        split_excess_waits(nc)
        _CACHE["nc"] = nc
    return _CACHE["nc"]


def _host_inputs(x, w_qkv, w_o):
    import ml_dtypes
    wq = np.ascontiguousarray(w_qkv.T).astype(np.float32).copy()
    wq[:, :D] *= np.float32(1.0 / math.sqrt(D))
    wo = np.ascontiguousarray(w_o.T).astype(np.float32)
    shared = {"wq": wq, "wo": wo}
    if CFG["qk"] == "bf16x3":
        wqk = wq[:, :2 * D]
        shared["wqh"] = wqk.astype(ml_dtypes.bfloat16)
        shared["wql"] = (wqk - shared["wqh"].astype(np.float32)).astype(
            ml_dtypes.bfloat16)
    in_maps = []
    for b in range(B):
        xT = np.ascontiguousarray(x[b].T)
        m = {"xT": xT, **shared}
        if CFG["qk"] == "bf16x3":
            m["xTh"] = xT.astype(ml_dtypes.bfloat16)
            m["xTl"] = (xT - m["xTh"].astype(np.float32)).astype(
                ml_dtypes.bfloat16)
        in_maps.append(m)
    return in_maps


def kernel(x, w_qkv, w_o, n_heads=H, **_unused):
    from concourse import bass_utils

    x = np.asarray(x, dtype=np.float32)
    w_qkv = np.asarray(w_qkv, dtype=np.float32)
    w_o = np.asarray(w_o, dtype=np.float32)
    assert int(n_heads) == H
    assert x.shape == (B, S, D), x.shape

    nc = _get_nc()
    in_maps = _host_inputs(x, w_qkv, w_o)
    res = bass_utils.run_bass_kernel_spmd(
        nc, in_maps, core_ids=list(range(N_CORES)))
    out = np.stack([res.results[b]["out"] for b in range(B)])
    return out.astype(np.float32)



# revision 27
# speedup vs baseline: 6.1317x; 6.1317x over previous
"""Trainium2 Bass kernel for the custom MHA problem nn_CustomMHA_14551349198906.

Computation (per batch b):
    t = x @ w_qkv.T ; q,k,v = heads of t        # (S, 3D), H=16 heads of 64
    scores = einsum('sid,sjd->sij', q, k)/sqrt(D)   # per-token 16x16
    lower-tri mask, exact-0 -> -inf, softmax over j
    y' = p @ v ; out = y' @ w_o.T

Distribution: pure data-parallel over the batch (B=8 -> one NeuronCore per
batch element); no collectives.

v2 design (per 128-token tile, tokens on partitions):
  - PE: all of t = x@w_qkv.T in single-pass f32r (1 cycle/row at 512-wide
    output, same rate as bf16; measured rel err 1.5e-2 < 2e-2 gate),
    y' transpose (fp16 identity matmul), out = y'@w_o.T in bf16.
  - scores (per-token 16x16, both operands per-token -> no PE): one
    f32 multiply + segmented reduce per diagonal (i-j); muls split
    DVE/Pool per POOL_MUL_DIAGS, reduces all on Pool (Pool's comparative
    advantage: reduce eff 0.6 vs mul eff 0.42) -> engine balance.
  - softmax on DVE (max/sub/sum/recip/norm) with exp on Act; score rect
    is a persistent pair with -3e38 pads memset once (reduces only ever
    write the valid triangle).
  - p@v in fp16: per-pow2-class batched multiplies (2x DVE mode: j
    innermost on all operands) + fold-halves add tree (in0/in1 are the
    two contiguous halves -> stays 2x until the last level), last level
    writes y' rows directly. TensorReduce has NO 2x mode, so the fold
    tree is ~2x cheaper than mul+reduce.

The local walrus build encodes at most ONE inline sync-wait per TPB
instruction; split_excess_waits() hoists extra waits onto same-engine NOPs.
"""

import math
from contextlib import ExitStack

import numpy as np

import bass_rust
import concourse.bass as bass
import concourse.mybir as mybir
import concourse.tile as tile
from concourse.masks import make_identity

F32 = mybir.dt.float32
F16 = mybir.dt.float16
BF16 = mybir.dt.bfloat16
F32R = mybir.dt.float32r

B, S, D, H, DH = 8, 4096, 1024, 16, 64
TILE = 128
KB = D // 128
N_TILES = S // TILE
NEG = -3.0e38
N_CORES = 8
W = 18   # padded softmax row width (W+1 = 19 diagonal stride)

CFG = {
    # diagonals whose score-mul runs on Pool (segmented reduces are
    # DVE-only: gpsimd tensor_reduce supports partition-axis only)
    "pool_mul_diags": tuple(range(16)),
    # softmax sub / normalize-mul engine: "pool" or "dve"
    "sub_eng": "pool",
    "norm_eng": "pool",
    # diagonals that get a second Pool fold (reduce reads cnt*16)
    "pool_l2_diags": (0, 1, 2),
    "bufs": 2,
}

# p@v pow2 classes: (i_start, i_count, padded nj); the nj=16 class is
# split in two i-halves sharing one product/fold buffer set (SBUF).
PV_CLASSES = [(8, 4, 16), (12, 4, 16), (4, 4, 8), (2, 2, 4), (1, 1, 2),
              (0, 1, 1)]


# --------------------------------------------------------------------------
# walrus workaround: hoist excess sync waits onto same-engine NOPs
# --------------------------------------------------------------------------
def split_excess_waits(nc, max_waits=1):
    n_split = 0
    for fn in nc.m.functions:
        for bb in fn.blocks:
            out = []
            changed = False
            for inst in bb.instructions:
                si = inst.sync_info
                waits = list(si.on_wait) if si is not None and si.on_wait else []
                if len(waits) > max_waits:
                    reg = [w for w in waits if getattr(w, "wait_reg", None) is not None]
                    imm = [w for w in waits if getattr(w, "wait_reg", None) is None]
                    kept = reg[:]
                    hoist = []
                    for w in imm:
                        if len(kept) < max_waits:
                            kept.append(w)
                        else:
                            hoist.append(w)
                    if len(kept) > max_waits:
                        raise RuntimeError(
                            f"{inst.name}: {len(reg)} register waits exceed limit")
                    for j, w in enumerate(hoist):
                        nop = mybir.InstNoOp(
                            name=f"{inst.name}-wsplit{j}",
                            ins=[], outs=[], engine=inst.engine)
                        nop.sync_info = bass_rust.SyncInfo(
                            on_wait=[w], on_update=[])
                        out.append(nop)
                        n_split += 1
                    inst.sync_info = bass_rust.SyncInfo(
                        on_wait=kept,
                        on_update=list(si.on_update) if si.on_update else [])
                    changed = True
                out.append(inst)
            if changed:
                bb.instructions = out
    return n_split


# --------------------------------------------------------------------------
# device program
# --------------------------------------------------------------------------
def _rearr_kb(ap):
    return ap.rearrange("(kb p) n -> p kb n", p=128)


def build(nc, cfg):
    bufs = cfg["bufs"]
    pool_mul = set(cfg["pool_mul_diags"])
    pool_l2 = set(cfg["pool_l2_diags"])
    sub_eng = cfg["sub_eng"]
    norm_eng = cfg["norm_eng"]

    xT = nc.dram_tensor("xT", (D, S), F32R, kind="ExternalInput").ap()
    wq = nc.dram_tensor("wq", (D, 3 * D), F32R, kind="ExternalInput").ap()
    wo = nc.dram_tensor("wo", (D, D), BF16, kind="ExternalInput").ap()
    out = nc.dram_tensor("out", (S, D), F32, kind="ExternalOutput").ap()

    with tile.TileContext(nc) as tc, ExitStack() as ctx:
        wpool = ctx.enter_context(tc.tile_pool(name="w", bufs=1))
        cpool = ctx.enter_context(tc.tile_pool(name="const", bufs=1))
        xpool = ctx.enter_context(tc.tile_pool(name="x", bufs=bufs))
        tpool = ctx.enter_context(tc.tile_pool(name="t", bufs=bufs))
        spool = ctx.enter_context(tc.tile_pool(name="sm", bufs=bufs))
        prpool = ctx.enter_context(tc.tile_pool(name="pr", bufs=bufs))
        pvpool = ctx.enter_context(tc.tile_pool(name="pv", bufs=1))
        ypool = ctx.enter_context(tc.tile_pool(name="y", bufs=1))
        opool = ctx.enter_context(tc.tile_pool(name="o", bufs=1))
        pp_t = ctx.enter_context(tc.tile_pool(name="ps_t", bufs=4, space="PSUM"))
        pp_tr = ctx.enter_context(tc.tile_pool(name="ps_tr", bufs=2, space="PSUM"))
        pp_o = ctx.enter_context(tc.tile_pool(name="ps_o", bufs=2, space="PSUM"))

        # chunked weight loads as separate tiles: the first t-matmul only
        # needs wq chunk 0, so the pipeline starts ~6us in instead of
        # after the full 38us DMA
        # preload the first two x tiles before any weight DMA so the SP
        # queue serves them first and the PE can start at ~6us
        xts = {}
        for n0 in range(2):
            xt0 = xpool.tile([128, KB, TILE], F32R, tag="xt", name=f"xt{n0}")
            nc.sync.dma_start(
                xt0[:], xT[:, n0 * TILE:(n0 + 1) * TILE].rearrange(
                    "(kb p) s -> p kb s", p=128))
            xts[n0] = xt0

        # each dma_start occupies the issuing engine's DMA queue for the
        # whole transfer -> spread the weight chunks over all idle queues
        wq_cs = []
        dma_engs = [nc.gpsimd, nc.scalar, nc.sync,
                    nc.gpsimd, nc.scalar, nc.sync]
        for oc in range(6):
            wq_c = wpool.tile([128, KB, 512], F32R, tag=f"wq{oc}",
                              name=f"wq{oc}")
            dma_engs[oc].dma_start(
                wq_c[:], _rearr_kb(wq)[:, :, oc * 512:(oc + 1) * 512])
            wq_cs.append(wq_c)
        wo_t = wpool.tile([128, KB, D], BF16, tag="wo")
        nc.sync.dma_start(wo_t[:], _rearr_kb(wo))

        ident = cpool.tile([128, 128], F16, tag="id")
        make_identity(nc, ident[:])

        # persistent score rects: pads memset to NEG once; per-tile reduces
        # only ever write the valid triangle positions.
        screbufs = []
        for kk in range(2):
            sc = cpool.tile([128, H * W], F32, tag=f"sc{kk}", name=f"screct{kk}")
            nc.gpsimd.memset(sc[:], NEG)
            screbufs.append(sc)

        for n in range(N_TILES):
            sl = slice(n * TILE, (n + 1) * TILE)

            if n in xts:
                xt = xts.pop(n)
            else:
                xt = xpool.tile([128, KB, TILE], F32R, tag="xt")
                nc.sync.dma_start(
                    xt[:], xT[:, sl].rearrange("(kb p) s -> p kb s", p=128))

            # ---- step 1: t = x @ w_qkv.T into 6 psum chunks -------------
            ps_t = []
            for oc in range(6):
                ps = pp_t.tile([128, 512], F32, tag="t", name=f"pst{n}_{oc}")
                for kb in range(KB):
                    nc.tensor.matmul(
                        ps[:], xt[:, kb, :], wq_cs[oc][:, kb, :],
                        start=(kb == 0), stop=(kb == KB - 1))
                ps_t.append(ps)

            # ---- drains -------------------------------------------------
            q_sb = tpool.tile([128, D], F32, tag="q")
            k_sb = tpool.tile([128, D], F32, tag="k")
            vt = tpool.tile([128, D], F16, tag="vt")
            nc.scalar.copy(q_sb[:, 0:512], ps_t[0][:])
            nc.scalar.copy(q_sb[:, 512:1024], ps_t[1][:])
            nc.scalar.copy(k_sb[:, 0:512], ps_t[2][:])
            nc.scalar.copy(k_sb[:, 512:1024], ps_t[3][:])
            # vt[:, 16*d + j] = v[:, 64*j + d]
            for h in range(2):
                src = ps_t[4 + h][:].rearrange("p (j d) -> p j d", d=DH)
                dst = vt[:].rearrange("p (d j) -> p d j", j=H)[
                    :, :, h * 8:(h + 1) * 8].rearrange("p d j -> p j d")
                nc.scalar.copy(dst, src)

            # ---- scores: one multiply + reduce per diagonal (i-j = o) ---
            screct = screbufs[n % 2]
            q3 = q_sb[:].rearrange("p (i d) -> p i d", d=DH)
            k3 = k_sb[:].rearrange("p (j d) -> p j d", d=DH)
            # per diagonal: Pool mul -> Pool fold (halve over d) -> DVE
            # segmented reduce (segmented reduces are DVE-only; the Pool
            # fold halves the DVE-side read volume).
            prod = prpool.tile([128, H * DH], F32, tag="prod", name="prod")
            phs = [prpool.tile([128, (H - b_) * (DH // 2)], F32,
                               tag=f"ph{b_}", name=f"ph{b_}")
                   for b_ in range(4)]
            pqs = [prpool.tile([128, (H - b_) * (DH // 4)], F32,
                               tag=f"pq{b_}", name=f"pq{b_}")
                   for b_ in range(2)]
            for o in range(H):
                cnt = H - o
                pr3 = prod[:, 0:cnt * DH].rearrange("p (j d) -> p j d", d=DH)
                meng = nc.gpsimd if o in pool_mul else nc.vector
                meng.tensor_mul(pr3, q3[:, o:H, :], k3[:, 0:cnt, :])
                ph = phs[o % 4]
                ph3 = ph[:, 0:cnt * (DH // 2)].rearrange(
                    "p (j d) -> p j d", d=DH // 2)
                meng.tensor_add(ph3, pr3[:, :, 0:DH // 2], pr3[:, :, DH // 2:])
                red_in = ph3
                if o in pool_l2:
                    pq = pqs[o % 2]
                    pq3 = pq[:, 0:cnt * (DH // 4)].rearrange(
                        "p (j d) -> p j d", d=DH // 4)
                    meng.tensor_add(
                        pq3, ph3[:, :, 0:DH // 4], ph3[:, :, DH // 4:])
                    red_in = pq3
                nc.vector.reduce_sum(
                    screct[:, W * o:W * o + (W + 1) * (cnt - 1) + 1:W + 1],
                    red_in, axis=mybir.AxisListType.X)

            # ---- softmax over j -----------------------------------------
            sc3 = screct[:].rearrange("p (i j) -> p i j", j=W)
            mrow = spool.tile([128, H], F32, tag="m")
            nc.vector.reduce_max(mrow[:], sc3, axis=mybir.AxisListType.X)
            mb = mrow[:].rearrange("p (i one) -> p i one", one=1).broadcast_to(
                (128, H, W))
            shifted = spool.tile([128, H * W], F32, tag="sh")
            (nc.gpsimd if sub_eng == "pool" else nc.vector).tensor_sub(
                shifted[:].rearrange("p (i j) -> p i j", j=W), sc3, mb)
            praw = spool.tile([128, H * W], F16, tag="e")
            nc.scalar.activation(praw[:], shifted[:],
                                 mybir.ActivationFunctionType.Exp)
            zrow = spool.tile([128, H], F32, tag="z")
            nc.vector.reduce_sum(
                zrow[:], praw[:].rearrange("p (i j) -> p i j", j=W),
                axis=mybir.AxisListType.X)
            rrow = spool.tile([128, H], F32, tag="r")
            nc.vector.reciprocal(rrow[:], zrow[:])
            prect = spool.tile([128, H * W], F16, tag="p")
            rb = rrow[:].rearrange("p (i one) -> p i one", one=1).broadcast_to(
                (128, H, W))
            (nc.gpsimd if norm_eng == "pool" else nc.vector).tensor_mul(
                prect[:].rearrange("p (i j) -> p i j", j=W),
                praw[:].rearrange("p (i j) -> p i j", j=W), rb)

            # ---- p @ v: batched pow2-class muls + fold-halves tree ------
            p3 = prect[:].rearrange("p (i j) -> p i j", j=W)
            vt3 = vt[:].rearrange("p (d j) -> p d j", j=H)
            yp = ypool.tile([128, D], F16, tag="yp")

            for (i0, ic, nj) in PV_CLASSES:
                # products pp[p, i, d, j] = prect[p, i0+i, j] * vt[p, d, j]
                if nj == 1:
                    dst = yp[:, i0 * DH:(i0 + ic) * DH].rearrange(
                        "p (i d one) -> p i d one", i=ic, one=1)
                else:
                    pc = pvpool.tile([128, ic * DH * nj], F16,
                                     tag=f"pc{ic}_{nj}", name=f"pc{ic}_{nj}")
                    dst = pc[:].rearrange(
                        "p (i d j) -> p i d j", i=ic, d=DH)
                pin = p3[:, i0:i0 + ic, 0:nj].rearrange(
                    "p i (one j) -> p i one j", one=1).broadcast_to(
                    (128, ic, DH, nj))
                vin = vt3[:, :, 0:nj].rearrange(
                    "p (one d) j -> p one d j", one=1).broadcast_to(
                    (128, ic, DH, nj))
                nc.vector.tensor_mul(dst, pin, vin)
                # fold-halves: m -> m/2 -> ... -> 1, last level into yp
                m = nj
                cur = dst
                while m > 1:
                    m //= 2
                    if m == 1:
                        nxt = yp[:, i0 * DH:(i0 + ic) * DH].rearrange(
                            "p (i d one) -> p i d one", i=ic, one=1)
                    else:
                        fbuf = pvpool.tile(
                            [128, ic * DH * m], F16, tag=f"f{ic}_{m}",
                            name=f"f{ic}_{m}")
                        nxt = fbuf[:].rearrange(
                            "p (i d j) -> p i d j", i=ic, d=DH)
                    feng = nc.gpsimd if (nj == 16 and m == 8) else nc.vector
                    feng.tensor_add(
                        nxt, cur[:, :, :, 0:m], cur[:, :, :, m:2 * m])
                    cur = nxt

            # ---- transpose y', then out = y' @ w_o.T --------------------
            ypT = ypool.tile([128, KB, TILE], BF16, tag="ypT")
            ps_tr = [pp_tr.tile([128, 512], F16, tag="tr", name=f"tr{n}_{h}")
                     for h in range(2)]
            for kb in range(KB):
                nc.tensor.transpose(
                    ps_tr[kb // 4][:, (kb % 4) * 128:(kb % 4 + 1) * 128],
                    yp[:, kb * 128:(kb + 1) * 128], ident[:])
            for h in range(2):
                nc.scalar.copy(
                    ypT[:].rearrange("p kb s -> p (kb s)")[
                        :, h * 512:(h + 1) * 512], ps_tr[h][:])

            osb = opool.tile([128, D], F32, tag="osb")
            for oc in range(2):
                ps_o = pp_o.tile([128, 512], F32, tag="o", name=f"pso{n}_{oc}")
                for kb in range(KB):
                    nc.tensor.matmul(
                        ps_o[:], ypT[:, kb, :], wo_t[:, kb, oc * 512:(oc + 1) * 512],
                        start=(kb == 0), stop=(kb == KB - 1))
                nc.scalar.copy(osb[:, oc * 512:(oc + 1) * 512], ps_o[:])
            nc.sync.dma_start(out[sl, :], osb[:])

    return nc


# --------------------------------------------------------------------------
# host side
# --------------------------------------------------------------------------
_CACHE = {}


def _get_nc():
    if "nc" not in _CACHE:
        nc = bass.Bass("TRN2", target_bir_lowering=False, debug=False,
                       num_devices=N_CORES)
        build(nc, CFG)
        split_excess_waits(nc)
        _CACHE["nc"] = nc
    return _CACHE["nc"]


def _host_inputs(x, w_qkv, w_o):
    import ml_dtypes
    wq = np.ascontiguousarray(w_qkv.T).astype(np.float32).copy()
    wq[:, :D] *= np.float32(1.0 / math.sqrt(D))
    wo = np.ascontiguousarray(w_o.T).astype(ml_dtypes.bfloat16)
    shared = {"wq": wq, "wo": wo}
    in_maps = []
    for b in range(B):
        xT = np.ascontiguousarray(x[b].T)
        in_maps.append({"xT": xT, **shared})
    return in_maps


def kernel(x, w_qkv, w_o, n_heads=H, **_unused):
    from concourse import bass_utils

    x = np.asarray(x, dtype=np.float32)
    w_qkv = np.asarray(w_qkv, dtype=np.float32)
    w_o = np.asarray(w_o, dtype=np.float32)
    assert int(n_heads) == H
    assert x.shape == (B, S, D), x.shape

    nc = _get_nc()
    in_maps = _host_inputs(x, w_qkv, w_o)
    res = bass_utils.run_bass_kernel_spmd(
        nc, in_maps, core_ids=list(range(N_CORES)))
    out = np.stack([res.results[b]["out"] for b in range(B)])
    return out.astype(np.float32)
